# revision 25
# baseline (speedup 1.0000x reference)
"""GRU layer kernel for Trainium2, data-parallel over 8 NeuronCores.

Strategy (feature-major / weight-stationary, bf16 + fp8-DoubleRow):
  - Shard batch N=2048 -> 8 cores x NB=256.
  - All matmul operands bf16 (1 col/cycle PE streaming, LDWEIGHTS 97ns
    fully hidden), except the z/r recurrent gemms (Wzh, Wrh) which run
    as fp8e4 DoubleRow: one instruction contracts K=256 (two 128-deep
    slots) in the same 107ns stream -> 2x FLOP rate on those gemms.
  - Weights pre-scaled by 1024 (exact power of 2; keeps fp8 tiles out
    of the subnormal range), descaled in the ACT with scale=2^-10.
  - h kept in bf16 pair tiles [128, 2, NB]; an fp8 copy (h8) feeds the
    DoubleRow matmuls. Precision (vs fp32 reference, CPU-simulated
    exactly): rel_err 1.6e-2 < 2e-2 gate.
  - Schedule: step t emits [zh/rh DR -> ACT r,z -> rh mul -> hh ->
    ACT g -> x-projections of t+1 (hoisted PE filler) -> blend -> h8],
    so the PE never waits on the recurrence chain. g-gate PSUM double
    buffered (zb 2 + rb 2 + gb 2x2 = 8 banks).
"""
import os
import numpy as np
import ml_dtypes

N, D = 2048, 512
T = int(os.environ.get("GRU_T", "64"))
NC = 8
NB = N // NC          # 256 batch rows per core
KT = D // 128         # 4 k-tiles
MT = D // 128         # 4 m-tiles
K2 = KT // 2          # 2 double-row k-tiles

WS = 1024.0           # weight pre-scale (exact power of 2)
DR_SET = set(filter(None, os.environ.get("GRU_DR", "zh,rh,rx").split(",")))

_CACHE = {}
LAST_RESULT = None

E4NP = ml_dtypes.float8_e4m3
BFNP = ml_dtypes.bfloat16

GEMMS = ("zx", "zh", "rx", "rh", "hx", "hh")


def _build_nc(zero_bias):
    import concourse.bacc as bacc
    import concourse.mybir as mybir
    from concourse.tile import TileContext, add_dep_helper

    f32 = mybir.dt.float32
    bf16 = mybir.dt.bfloat16
    fp8 = mybir.dt.float8e4
    DRM = mybir.MatmulPerfMode.DoubleRow
    Sig = mybir.ActivationFunctionType.Sigmoid
    Tanh = mybir.ActivationFunctionType.Tanh
    SCL = 1.0 / WS

    nc = bacc.Bacc("TRN2", target_bir_lowering=False, debug=False, num_devices=NC)

    xt_d = nc.dram_tensor("xt", [T, D, NB], bf16, kind="ExternalInput")
    x8_needed = bool(DR_SET & {"zx", "rx", "hx"})
    if x8_needed:
        xt8_d = nc.dram_tensor("xt8", [T, D, NB], fp8, kind="ExternalInput")
    w_d = {}
    for g in GEMMS:
        if g in DR_SET:
            w_d[g] = nc.dram_tensor(f"w{g}", [128, 2, K2 * MT * 128], fp8,
                                    kind="ExternalInput")
        else:
            w_d[g] = nc.dram_tensor(f"w{g}", [128, KT * MT * 128], bf16,
                                    kind="ExternalInput")
    b_d = {b: nc.dram_tensor(b, [128, MT], f32, kind="ExternalInput")
           for b in ("bz", "br", "bh")}
    out_d = nc.dram_tensor("out", [T, D, NB], bf16, kind="ExternalOutput")

    with TileContext(nc) as tc:
        with (
            tc.tile_pool(name="wsb", bufs=1) as wsb,
            tc.tile_pool(name="xsb", bufs=4) as xsb,
            tc.tile_pool(name="ssb", bufs=2) as ssb,
            tc.tile_pool(name="hsb", bufs=3) as hsb,
            tc.tile_pool(name="psum", bufs=1, space="PSUM") as psum,
            tc.tile_pool(name="psum2", bufs=2, space="PSUM") as psum2,
        ):
            w_sb = {}
            for g in GEMMS:
                if g in DR_SET:
                    w_sb[g] = wsb.tile([128, 2, K2 * MT * 128], fp8, name=f"w_{g}")
                else:
                    w_sb[g] = wsb.tile([128, KT * MT * 128], bf16, name=f"w_{g}")
            b_sb = {b: wsb.tile([128, MT], f32, name=f"b_{b}") for b in b_d}

            def wdma(g, nchunks=2):
                insts = []
                if g in DR_SET:
                    cw = K2 * MT * 128 // nchunks
                    for u in range(nchunks):
                        insts.append(nc.sync.dma_start(
                            out=w_sb[g][:, :, u * cw:(u + 1) * cw],
                            in_=w_d[g][:, :, u * cw:(u + 1) * cw]))
                else:
                    cw = KT * MT * 128 // nchunks
                    for u in range(nchunks):
                        insts.append(nc.sync.dma_start(
                            out=w_sb[g][:, u * cw:(u + 1) * cw],
                            in_=w_d[g][:, u * cw:(u + 1) * cw]))
                return insts

            def xdma(t, with8=False):
                # split across 2 DMA queues to halve arrival latency
                xt = xsb.tile([128, KT, NB], bf16, name=f"x{t}", tag="xt")
                src = xt_d[t].rearrange("(k p) b -> p k b", p=128)
                d = nc.sync.dma_start(out=xt[:, :2, :], in_=src[:, :2, :])
                nc.sync.dma_start(out=xt[:, 2:, :], in_=src[:, 2:, :])
                if not (with8 and x8_needed):
                    return xt, d
                x8 = xsb.tile([128, K2, 2, NB], fp8, name=f"x8{t}", tag="xt8")
                src8 = xt8_d[t].rearrange("(k2 i p) b -> p k2 i b", p=128, i=2)
                nc.sync.dma_start(out=x8[:, 0], in_=src8[:, 0])
                nc.sync.dma_start(out=x8[:, 1], in_=src8[:, 1])
                return (xt, x8), d

            # staged startup DMAs: zx weights + x0 first (t=0's first MMs),
            # then hx, then rx, then the recurrent weights
            pri0 = []
            pri0 += wdma("zx", 8)
            pri0.append(nc.sync.dma_start(out=b_sb["bz"][:], in_=b_d["bz"][:]))
            xt0, d0 = xdma(0)
            pri0.append(d0)

            pri1 = []
            pri1 += wdma("hx", 4)
            pri1.append(nc.sync.dma_start(out=b_sb["bh"][:], in_=b_d["bh"][:]))

            pri2 = wdma("rx", 4)

            late = []
            late += wdma("zh")
            late += wdma("rh")
            late += wdma("hh")
            late.append(nc.sync.dma_start(out=b_sb["br"][:], in_=b_d["br"][:]))

            def gate(group, anchors, why):
                for li in group:
                    for pi in anchors:
                        add_dep_helper(li.ins, pi.ins, sync=True, reason=why)

            gate(pri1, (pri0[6], pri0[7], pri0[-1]), "startup: hx after zx")
            gate(pri2, (pri1[2], pri1[3]), "startup: rx after hx")
            gate(late, (pri2[2], pri2[3]), "startup: recurrent last")

            def wtile(g, k, mi):
                off = (k * MT + mi) * 128
                return w_sb[g][:, off:off + 128]

            def wtile8(g, k2, mi):
                off = (k2 * MT + mi) * 128
                return w_sb[g][:, :, off:off + 128]

            # psum banks: zb/rb single-buffered, gb double-buffered
            def zbank(b):
                return psum.tile([128, 512], f32, name=f"zb{b}", tag=f"zb{b}")

            def rbank(b):
                return psum.tile([128, 512], f32, name=f"rb{b}", tag=f"rb{b}")

            def gbank(b):
                return psum2.tile([128, 512], f32, name=f"gb{b}", tag=f"gb{b}")

            def half(banks, mi):
                return banks[mi // 2][:, (mi % 2) * NB:(mi % 2 + 1) * NB]

            # x-projection of step t into the given banks (start of group).
            # order="k" emits k-outer (prologue: MMs can start after the
            # first weight-DMA chunk arrives). start/stop flags hold for
            # either order: first MM into a bank is (k==0, mi even), last
            # is (k==KT-1, mi odd).
            def xproj(g, banks, xt, x8=None, stop_at_x=False, order="m"):
                if g in DR_SET:
                    for mi in range(MT):
                        for k2 in range(K2):
                            nc.tensor.matmul(
                                half(banks, mi), wtile8(g, k2, mi),
                                x8[:, k2, :, :],
                                start=(mi % 2 == 0 and k2 == 0),
                                stop=(stop_at_x and mi % 2 == 1 and k2 == K2 - 1),
                                perf_mode=DRM)
                else:
                    loop = ([(mi, k) for mi in range(MT) for k in range(KT)]
                            if order == "m" else
                            [(mi, k) for k in range(KT) for mi in range(MT)])
                    for mi, k in loop:
                        nc.tensor.matmul(
                            half(banks, mi), wtile(g, k, mi), xt[:, k, :],
                            start=(mi % 2 == 0 and k == 0),
                            stop=(stop_at_x and mi % 2 == 1 and k == KT - 1))

            # recurrent part of a z/r gate (accumulate + stop)
            def hpart(g, banks, h8p, hbp):
                if g in DR_SET:
                    for mi in range(MT):
                        for k2 in range(K2):
                            nc.tensor.matmul(
                                half(banks, mi), wtile8(g, k2, mi), h8p[k2][:],
                                start=False,
                                stop=(mi % 2 == 1 and k2 == K2 - 1),
                                perf_mode=DRM)
                else:
                    for mi in range(MT):
                        for k in range(KT):
                            nc.tensor.matmul(
                                half(banks, mi), wtile(g, k, mi),
                                hbp[k // 2][:, k % 2, :],
                                start=False,
                                stop=(mi % 2 == 1 and k == KT - 1))

            # activations: bank-level if biases are zero, else per m-tile
            def act(dst_pairs, banks, fn, bname, dt_):
                if zero_bias:
                    for b in range(2):
                        nc.scalar.activation(dst_pairs[b][:], banks[b][:],
                                             fn, scale=SCL)
                else:
                    for mi in range(MT):
                        nc.scalar.activation(
                            dst_pairs[mi // 2][:, mi % 2, :], half(banks, mi),
                            fn, bias=b_sb[bname][:, mi:mi + 1], scale=SCL)

            def pairs(tag, dt_):
                return [ssb.tile([128, 2, NB], dt_, name=f"{tag}{b}",
                                 tag=f"{tag}{b}") for b in range(2)]

            DR_ANY = bool(DR_SET)
            h8_prev = None
            hb_prev = None
            xt_next = None

            # ---- t = 0 prologue: h0 = (1-z)*g, no r gate
            zb = [zbank(b) for b in range(2)]
            gb = [gbank(b) for b in range(2)]
            xproj("zx", zb, xt0, stop_at_x=True, order="k")
            xproj("hx", gb, xt0, stop_at_x=True, order="k")
            xt1p, _ = xdma(1, with8=True)
            xt1, x81 = xt1p if x8_needed else (xt1p, None)
            z_p = pairs("z", f32)
            g_p = pairs("g", f32)
            act(z_p, zb, Sig, "bz", f32)
            act(g_p, gb, Tanh, "bh", f32)
            hb = [hsb.tile([128, 2, NB], bf16, name=f"h0p{b}", tag=f"hb{b}")
                  for b in range(2)]
            h8 = ([hsb.tile([128, 2, NB], fp8, name=f"h80p{b}", tag=f"h8{b}")
                   for b in range(2)] if DR_ANY else None)
            for p in range(2):
                tmp = ssb.tile([128, 2, NB], f32, name=f"tm0p{p}", tag=f"tmp{p}")
                nc.vector.tensor_mul(tmp[:], z_p[p][:], g_p[p][:])
                nc.vector.tensor_sub(hb[p][:], g_p[p][:], tmp[:])
                nc.sync.dma_start(
                    out=out_d[0, p * 256:(p + 1) * 256, :].rearrange(
                        "(i q) b -> q i b", q=128),
                    in_=hb[p][:])
            if DR_ANY:
                nc.scalar.copy(h8[0][:], hb[0][:])
                nc.vector.tensor_copy(h8[1][:], hb[1][:])
            hb_prev, h8_prev = hb, h8
            # hoist x-projections of t=1 (hx first: gb is double-buffered
            # so it has no write-after-read wait on the activations)
            gb = [gbank(b) for b in range(2)]
            xproj("hx", gb, xt1, x81)
            rb = [rbank(b) for b in range(2)]
            xproj("zx", zb, xt1, x81)
            xproj("rx", rb, xt1, x81)
            # prefetch x of t=2 (consumed by body t=1's hoist); the body
            # prefetches t+2 so x arrives a full step before first use
            if T > 2:
                xt2p, _ = xdma(2, with8=True)
                xt_next, x8_next = xt2p if x8_needed else (xt2p, None)

            # ---- steady-state steps
            for t in range(1, T):
                if t + 2 < T:
                    xtp, _ = xdma(t + 2, with8=True)
                    xt_new, x8_new = xtp if x8_needed else (xtp, None)

                # recurrent z/r parts (x-parts already accumulated)
                hpart("rh", rb, h8_prev, hb_prev)
                hpart("zh", zb, h8_prev, hb_prev)

                r_p = pairs("r", bf16)
                act(r_p, rb, Sig, "br", bf16)
                z_p = pairs("z", f32)
                g_p = pairs("g", f32)
                if zero_bias:
                    # interleave ACT FIFO as r0,r1,z0,g0,z1,g1 so g0 isn't
                    # queued behind both z activations
                    nc.scalar.activation(z_p[0][:], zb[0][:], Sig, scale=SCL)
                else:
                    act(z_p, zb, Sig, "bz", f32)

                # rh = r * h  (bf16 pairs, moving operand of hh)
                rh_p = pairs("rh", bf16)
                for p in range(2):
                    nc.vector.tensor_mul(rh_p[p][:], r_p[p][:], hb_prev[p][:])

                # hh: bank-major, k2-paired, so g bank0 completes after 8 MMs
                # and rh pair1 is first needed at MM #5
                for b in range(2):
                    for kp in range(K2):
                        for mi in (2 * b, 2 * b + 1):
                            for k in (2 * kp, 2 * kp + 1):
                                nc.tensor.matmul(
                                    half(gb, mi), wtile("hh", k, mi),
                                    rh_p[k // 2][:, k % 2, :],
                                    start=False,
                                    stop=(k == KT - 1 and mi == 2 * b + 1))

                if zero_bias:
                    # g1 before z1: z1 is first needed by the pair-1 blend
                    # mul, well after the pair-1 sub that needs g1
                    nc.scalar.activation(g_p[0][:], gb[0][:], Tanh, scale=SCL)
                    nc.scalar.activation(g_p[1][:], gb[1][:], Tanh, scale=SCL)
                    nc.scalar.activation(z_p[1][:], zb[1][:], Sig, scale=SCL)
                else:
                    act(g_p, gb, Tanh, "bh", f32)

                # hoisted x-projections of step t+1 (PE filler; hx first —
                # gb double-buffered, no WAR wait on this step's ACTs)
                if t + 1 < T:
                    gb_next = [gbank(b) for b in range(2)]
                    xproj("hx", gb_next, xt_next, x8_next)
                    zb = [zbank(b) for b in range(2)]
                    rb = [rbank(b) for b in range(2)]
                    xproj("zx", zb, xt_next, x8_next)
                    xproj("rx", rb, xt_next, x8_next)
                    gb = gb_next
                    if t + 2 < T:
                        xt_next, x8_next = xt_new, x8_new

                # blend: h = g + (h_prev - g) * z, pair-level, bf16 out
                hb = [hsb.tile([128, 2, NB], bf16, name=f"h{t}p{b}",
                               tag=f"hb{b}") for b in range(2)]
                h8 = ([hsb.tile([128, 2, NB], fp8, name=f"h8{t}p{b}",
                                tag=f"h8{b}") for b in range(2)]
                      if DR_ANY else None)
                for p in range(2):
                    tmp = ssb.tile([128, 2, NB], f32, name=f"tm{t}p{p}",
                                   tag=f"tmp{p}")
                    nc.vector.tensor_sub(tmp[:], hb_prev[p][:], g_p[p][:])
                    nc.vector.tensor_mul(tmp[:], tmp[:], z_p[p][:])
                    nc.vector.tensor_add(hb[p][:], g_p[p][:], tmp[:])
                    if DR_ANY and p == 0:
                        nc.scalar.copy(h8[0][:], hb[0][:])
                    if DR_ANY and p == 1:
                        nc.vector.tensor_copy(h8[1][:], hb[1][:])
                    nc.sync.dma_start(
                        out=out_d[t, p * 256:(p + 1) * 256, :].rearrange(
                            "(i q) b -> q i b", q=128),
                        in_=hb[p][:])
                hb_prev, h8_prev = hb, h8

    nc.compile()
    return nc


def _get_nc(zero_bias):
    key = (tuple(sorted(DR_SET)), zero_bias, T)
    if key not in _CACHE:
        _CACHE[key] = _build_nc(zero_bias)
    return _CACHE[key]


def _pack_wb(W):
    # bf16 lhsT tiles [128, KT*MT*128], tile (k,m) at offset (k*MT+m)*128:
    # w[p, off+q] = W[m*128+q, k*128+p] * WS
    Wt = (np.asarray(W, np.float32).T * WS).reshape(KT, 128, MT, 128)
    return np.ascontiguousarray(
        Wt.transpose(1, 0, 2, 3).reshape(128, KT * MT * 128).astype(BFNP))


def _pack_w8(W):
    # fp8 DoubleRow tiles [128, 2, K2*MT*128], tile (k2,m):
    # w[p, i, off+q] = W[m*128+q, (2*k2+i)*128+p] * WS
    Wt = (np.asarray(W, np.float32).T * WS).reshape(K2, 2, 128, MT, 128)
    return np.ascontiguousarray(
        Wt.transpose(2, 1, 0, 3, 4).reshape(128, 2, K2 * MT * 128).astype(E4NP))


def kernel(inputss, Wzx, Wzh, Wrx, Wrh, Whx, Whh, bz, br, bh):
    global LAST_RESULT
    from concourse.bass_utils import run_bass_kernel_spmd

    inputss = np.asarray(inputss, np.float32)
    assert inputss.shape == (N, T, D), inputss.shape

    Wmap = {"zx": Wzx, "zh": Wzh, "rx": Wrx, "rh": Wrh, "hx": Whx, "hh": Whh}
    bmap = {"bz": bz, "br": br, "bh": bh}
    zero_bias = all(not np.any(np.asarray(b)) for b in bmap.values())

    xs = inputss.reshape(NC, NB, T, D).transpose(0, 2, 3, 1)  # [NC, T, D, NB]
    xs = np.ascontiguousarray(xs.astype(BFNP))
    x8_needed = bool(DR_SET & {"zx", "rx", "hx"})
    if x8_needed:
        xs8 = np.ascontiguousarray(xs.astype(np.float32).astype(E4NP))
    wp = {f"w{g}": (_pack_w8(Wmap[g]) if g in DR_SET else _pack_wb(Wmap[g]))
          for g in GEMMS}
    bp = {k: np.ascontiguousarray(np.asarray(v, np.float32).reshape(MT, 128).T)
          for k, v in bmap.items()}

    in_maps = []
    for c in range(NC):
        m = {"xt": xs[c]}
        if x8_needed:
            m["xt8"] = xs8[c]
        m.update(wp)
        m.update(bp)
        in_maps.append(m)

    nc = _get_nc(zero_bias)
    trace = bool(int(os.environ.get("GRU_TRACE", "0")))
    res = run_bass_kernel_spmd(nc, in_maps, core_ids=list(range(NC)), trace=trace)
    LAST_RESULT = res

    outs = np.stack([np.asarray(res.results[c]["out"]).astype(np.float32)
                     for c in range(NC)])  # [NC, T, D, NB]
    return np.ascontiguousarray(outs.transpose(0, 3, 1, 2).reshape(N, T, D))
